# revision 5
# baseline (speedup 1.0000x reference)
"""ChannelAttentionModule Trainium2 kernel.

Reference computation (B=128, C=704, L=1024, G=11 groups of GW=64 channels):
    y_avg = mean(x, -1); y_max = max(x, -1)                      # [B, C]
    gate  = sigmoid(mlp(y_avg) + mlp(y_max))                     # [B, C]
    out   = x * gate[:, :, None]
where mlp is a per-group linear pair (W1[g]: 64x16, W2[g]: 16x64) with NO
nonlinearity between them, so mlp(a) + mlp(b) = a @ Wc + b @ Wc with
Wc[g] = W1[g] @ W2[g] (64x64), and mean = sum/L can be folded into a
pre-scaled copy of Wc.

Sharding: data-parallel on batch across 8 cores (16 batches/core). Two
consecutive batches = 2*704 = 1408 rows = exactly 11 tiles of 128 rows, and
each 64-row half-tile is one complete (batch, group) channel block, so every
[128, 1024] tile's gate depends only on that tile's own row stats:
    load tile -> reduce_sum + reduce_max (free dim) -> 2 accumulating
    matmuls against a 128x128 block-diagonal stationary weight -> sigmoid
    -> per-partition scaled copy -> store.
"""

import os
import sys

import numpy as np

for _p in ("/opt/trn_rl_repo", "/root/.axon_site/_ro/trn_rl_repo"):
    if os.path.isdir(_p) and _p not in sys.path:
        sys.path.insert(0, _p)

import concourse.bacc as bacc
import concourse.bass as bass
import concourse.tile as tile
from concourse import mybir
from concourse.bass_utils import run_bass_kernel_spmd

B, C, L = 128, 704, 1024
G, GW = 11, 64
NCORES = 8
BPC = B // NCORES            # batches per core = 16
NPAIRS = BPC // 2            # 8
PAIR_ROWS = 2 * C            # 1408
NTILES = PAIR_ROWS // 128    # 11
ROWS = BPC * C               # 11264
F32 = mybir.dt.float32

_PROGRAM = None


def _build_program(npairs=NPAIRS):
    # PE Matmult lowering (walrus S3_LW) only tolerates ONE sync wait per
    # matmul, so every PE dependency is funneled through the DVE semaphore:
    # the weight tile is re-copied by DVE, the combined stat is written by
    # DVE, and the PSUM slot is released by a DVE copy.
    nc = bacc.Bacc(None)
    x = nc.declare_dram_parameter("x", [npairs * PAIR_ROWS, L], F32, isOutput=False)
    w = nc.declare_dram_parameter("W", [128, NTILES * 128], F32, isOutput=False)
    out = nc.declare_dram_parameter("out", [npairs * PAIR_ROWS, L], F32, isOutput=True)

    with tile.TileContext(nc) as tc:
        with (
            tc.tile_pool(name="singles", bufs=1) as singles,
            tc.tile_pool(name="xp", bufs=8) as xp,
            tc.tile_pool(name="small", bufs=8) as small,
            tc.tile_pool(name="psum", bufs=8, space=bass.MemorySpace.PSUM) as psums,
        ):
            wt_raw = singles.tile([128, NTILES * 128], F32)
            nc.sync.dma_start(out=wt_raw, in_=w[:, :])
            wt = singles.tile([128, NTILES * 128], F32)
            nc.vector.tensor_copy(out=wt, in_=wt_raw)

            for p in range(npairs):
                for t in range(NTILES):
                    r0 = p * PAIR_ROWS + t * 128
                    xt = xp.tile([128, L], F32)
                    nc.sync.dma_start(out=xt, in_=x[r0 : r0 + 128, :])

                    s = small.tile([128, 1], F32, tag="s")
                    m = small.tile([128, 1], F32, tag="m")
                    nc.vector.reduce_sum(out=s, in_=xt, axis=mybir.AxisListType.X)
                    nc.vector.reduce_max(out=m, in_=xt, axis=mybir.AxisListType.X)
                    comb = small.tile([128, 1], F32, tag="c")
                    nc.vector.tensor_scalar(
                        out=comb, in0=s, scalar1=1.0 / L, scalar2=m,
                        op0=mybir.AluOpType.mult, op1=mybir.AluOpType.add,
                    )

                    pc = psums.tile([128, 1], F32)
                    nc.tensor.matmul(
                        pc, wt[:, t * 128 : (t + 1) * 128], comb,
                        start=True, stop=True,
                    )
                    gsb = small.tile([128, 1], F32, tag="o")
                    nc.vector.tensor_copy(out=gsb, in_=pc)

                    gate = small.tile([128, 1], F32, tag="g")
                    nc.scalar.activation(
                        out=gate, in_=gsb, func=mybir.ActivationFunctionType.Sigmoid
                    )
                    nc.scalar.mul(out=xt, in_=xt, mul=gate)
                    nc.sync.dma_start(out=out[r0 : r0 + 128, :], in_=xt)
    if not nc.is_finalized():
        nc.finalize()
    return nc


def _pack_weights(W1, W2):
    # Wc[g] = W1[g] @ W2[g]; tile t holds blocks 2t (partitions 0:64) and
    # 2t+1 (partitions 64:128); block k -> group k % 11. The 1/L mean scale
    # is applied on DVE when combining sum+max, so weights are unscaled.
    Wc = np.einsum(
        "gch,ghd->gcd", W1.astype(np.float64), W2.astype(np.float64)
    ).astype(np.float32)
    wpk = np.zeros((128, NTILES, 128), np.float32)
    for t in range(NTILES):
        gt, gb = (2 * t) % G, (2 * t + 1) % G
        wpk[0:64, t, 0:64] = Wc[gt]
        wpk[64:128, t, 64:128] = Wc[gb]
    return wpk.reshape(128, NTILES * 128)


def _get_program():
    global _PROGRAM
    if _PROGRAM is None:
        _PROGRAM = _build_program()
    return _PROGRAM


def run(x, W1, W2, trace=False, **kwargs):
    nc = _get_program()
    wpk = _pack_weights(np.asarray(W1), np.asarray(W2))
    xs = np.ascontiguousarray(x).reshape(NCORES, ROWS, L)
    in_maps = [{"x": xs[i], "W": wpk} for i in range(NCORES)]
    res = run_bass_kernel_spmd(
        nc, in_maps, core_ids=list(range(NCORES)), trace=trace, **kwargs
    )
    out = np.empty((NCORES, ROWS, L), np.float32)
    for i in range(NCORES):
        out[i] = res.results[i]["out"]
    return out.reshape(B, C, L), res


def kernel(x, W1, W2):
    out, _ = run(x, W1, W2)
    return out


# revision 7
# speedup vs baseline: 1.1107x; 1.1107x over previous
"""ChannelAttentionModule Trainium2 kernel.

Reference computation (B=128, C=704, L=1024, G=11 groups of GW=64 channels):
    y_avg = mean(x, -1); y_max = max(x, -1)                      # [B, C]
    gate  = sigmoid(mlp(y_avg) + mlp(y_max))                     # [B, C]
    out   = x * gate[:, :, None]
where mlp is a per-group linear pair (W1[g]: 64x16, W2[g]: 16x64) with NO
nonlinearity between them, so mlp(a) + mlp(b) = a @ Wc + b @ Wc with
Wc[g] = W1[g] @ W2[g] (64x64), and mean = sum/L can be folded into a
pre-scaled copy of Wc.

Sharding: data-parallel on batch across 8 cores (16 batches/core). Two
consecutive batches = 2*704 = 1408 rows = exactly 11 tiles of 128 rows, and
each 64-row half-tile is one complete (batch, group) channel block, so every
[128, 1024] tile's gate depends only on that tile's own row stats:
    load tile -> reduce_sum + reduce_max (free dim) -> 2 accumulating
    matmuls against a 128x128 block-diagonal stationary weight -> sigmoid
    -> per-partition scaled copy -> store.
"""

import os
import sys

import numpy as np

for _p in ("/opt/trn_rl_repo", "/root/.axon_site/_ro/trn_rl_repo"):
    if os.path.isdir(_p) and _p not in sys.path:
        sys.path.insert(0, _p)

import concourse.bacc as bacc
import concourse.bass as bass
import concourse.tile as tile
from concourse import mybir
from concourse.bass_utils import run_bass_kernel_spmd

B, C, L = 128, 704, 1024
G, GW = 11, 64
NCORES = 8
BPC = B // NCORES            # batches per core = 16
NPAIRS = BPC // 2            # 8
PAIR_ROWS = 2 * C            # 1408
NTILES = PAIR_ROWS // 128    # 11
ROWS = BPC * C               # 11264
F32 = mybir.dt.float32

_PROGRAM = None


def _build_program(npairs=NPAIRS, blk=4, xbufs=6, sbufs=16):
    # blk row-tiles ride in each DMA (blk*512KB transfers) to amortize DMA
    # fixed cost; per 128-row subtile the chain is
    #   reduce_sum+reduce_max+combine (DVE) -> matmul vs block-diag weight
    #   (PE) -> psum->sbuf copy (DVE) -> sigmoid (ACT) -> scaled copy (ACT,
    #   in place) -> store.
    nc = bacc.Bacc(None)
    rows = npairs * PAIR_ROWS
    ntile = rows // 128
    assert ntile % blk == 0
    x = nc.declare_dram_parameter("x", [rows, L], F32, isOutput=False)
    w = nc.declare_dram_parameter("W", [128, NTILES * 128], F32, isOutput=False)
    out = nc.declare_dram_parameter("out", [rows, L], F32, isOutput=True)
    xr = x[:, :].rearrange("(n a p) l -> n p a l", a=blk, p=128)
    outr = out[:, :].rearrange("(n a p) l -> n p a l", a=blk, p=128)

    with tile.TileContext(nc) as tc:
        with (
            tc.tile_pool(name="singles", bufs=1) as singles,
            tc.tile_pool(name="xp", bufs=xbufs) as xp,
            tc.tile_pool(name="small", bufs=sbufs) as small,
            tc.tile_pool(name="psum", bufs=8, space=bass.MemorySpace.PSUM) as psums,
        ):
            wt_raw = singles.tile([128, NTILES * 128], F32)
            nc.sync.dma_start(out=wt_raw, in_=w[:, :])
            wt = singles.tile([128, NTILES * 128], F32)
            nc.vector.tensor_copy(out=wt, in_=wt_raw)

            for n in range(ntile // blk):
                xt = xp.tile([128, blk, L], F32)
                nc.sync.dma_start(out=xt, in_=xr[n])
                for a in range(blk):
                    t = (n * blk + a) % NTILES
                    xs = xt[:, a, :]
                    s = small.tile([128, 1], F32, tag="s")
                    m = small.tile([128, 1], F32, tag="m")
                    nc.vector.reduce_sum(out=s, in_=xs, axis=mybir.AxisListType.X)
                    nc.vector.reduce_max(out=m, in_=xs, axis=mybir.AxisListType.X)
                    comb = small.tile([128, 1], F32, tag="c")
                    nc.vector.tensor_scalar(
                        out=comb, in0=s, scalar1=1.0 / L, scalar2=m,
                        op0=mybir.AluOpType.mult, op1=mybir.AluOpType.add,
                    )

                    pc = psums.tile([128, 1], F32)
                    nc.tensor.matmul(
                        pc, wt[:, t * 128 : (t + 1) * 128], comb,
                        start=True, stop=True,
                    )
                    gsb = small.tile([128, 1], F32, tag="o")
                    nc.vector.tensor_copy(out=gsb, in_=pc)

                    gate = small.tile([128, 1], F32, tag="g")
                    nc.scalar.activation(
                        out=gate, in_=gsb, func=mybir.ActivationFunctionType.Sigmoid
                    )
                    nc.scalar.mul(out=xs, in_=xs, mul=gate)
                nc.sync.dma_start(out=outr[n], in_=xt)
    if not nc.is_finalized():
        nc.finalize()
    return nc


def _pack_weights(W1, W2):
    # Wc[g] = W1[g] @ W2[g]; tile t holds blocks 2t (partitions 0:64) and
    # 2t+1 (partitions 64:128); block k -> group k % 11. The 1/L mean scale
    # is applied on DVE when combining sum+max, so weights are unscaled.
    Wc = np.einsum(
        "gch,ghd->gcd", W1.astype(np.float64), W2.astype(np.float64)
    ).astype(np.float32)
    wpk = np.zeros((128, NTILES, 128), np.float32)
    for t in range(NTILES):
        gt, gb = (2 * t) % G, (2 * t + 1) % G
        wpk[0:64, t, 0:64] = Wc[gt]
        wpk[64:128, t, 64:128] = Wc[gb]
    return wpk.reshape(128, NTILES * 128)


def _get_program():
    global _PROGRAM
    if _PROGRAM is None:
        _PROGRAM = _build_program()
    return _PROGRAM


def run(x, W1, W2, trace=False, **kwargs):
    nc = _get_program()
    wpk = _pack_weights(np.asarray(W1), np.asarray(W2))
    xs = np.ascontiguousarray(x).reshape(NCORES, ROWS, L)
    in_maps = [{"x": xs[i], "W": wpk} for i in range(NCORES)]
    res = run_bass_kernel_spmd(
        nc, in_maps, core_ids=list(range(NCORES)), trace=trace, **kwargs
    )
    out = np.empty((NCORES, ROWS, L), np.float32)
    for i in range(NCORES):
        out[i] = res.results[i]["out"]
    return out.reshape(B, C, L), res


def kernel(x, W1, W2):
    out, _ = run(x, W1, W2)
    return out
